# revision 23
# baseline (speedup 1.0000x reference)
"""Trainium2 SPMD kernel for nn_AutoregressiveDecoder (gnn_message_passing).

Math (reference, per context g in 0..N-1, N=384):
    h1[g]  = concat(z, e_g) @ W1 = H0 + e_g (x) W1r     # H0 = z @ W1[:128]
    A[g]   = relu(P_g @ h1[g])         P_g = partials[g]
    h2[g]  = A[g] @ W2
    h3[g]  = P_g @ h2[g]
    S[g,:] = h3[g][g,:] @ h3[g].T      (row g of supplement, pre-tril)
    out    = x + 0.5*(tril(S) + tril(S).T)

8 cores x 48 contexts, raw Bass (manual semaphores), fp32r matmuls except
the A@W2 stage in bf16.  Per context i (software-pipelined, skew 3):
    mm1  A_T[h,:]  = sum_j H0[j,h] Pt[j,:]  (+ rank-1 W1r (x) pcol)  N=384
    mm2  h2[j,k]   = sum_h A_T[h,j] W2[h,k]                          N=128 bf16
    mm3  h3T[k,:]  = sum_j h2[j,k] PtAug[j,:]  (col 384 = d vector)  N=385
    mm4  S[1,:]    = sum_k d[k] h3T[k,:]   (psum row aliased in h3ps) N=384
Pt = P_g.T pre-transposed on host; PtAug's col 384 is P_g[g,:] so mm3 also
yields d = h3[g][g,:].  tril/symmetrize/(+x) happen on host at unshard.
PE stream at iter i: mm1(i), mm2(i-1), mm3(i-2), mm4(i-3) -- the ACT/DVE
relu/copies of stage k run a full iteration before their PE consumer.
"""

import os
from contextlib import ExitStack

import numpy as np
import ml_dtypes

import concourse.bass as bass
import concourse.mybir as mybir
from concourse.bass_utils import run_bass_kernel_spmd

N = 384
D = 128
HID = 256
HID2 = 128
NCORES = 8
NB = N // NCORES  # 48 contexts per core
W = N + 2  # pt width: prow column at N, plus even-size pad (fp32r dst rule)
PTBUF = 6  # pt SBUF ring depth
SRBUF = 8  # S-row SBUF ring depth

F32 = mybir.dt.float32
F32R = mybir.dt.float32r
BF16 = mybir.dt.bfloat16
AFT = mybir.ActivationFunctionType

_NC_CACHE = {}
LAST_RESULT = None  # test.py reads exec_time_ns from here


def _round_f32r(a: np.ndarray) -> np.ndarray:
    """Round fp32 to fp32r (TF32-like: low 12 mantissa bits cleared, RNE)."""
    u = np.ascontiguousarray(a, dtype=np.float32).view(np.uint32)
    add = np.uint32(0x7FF) + ((u >> np.uint32(12)) & np.uint32(1))
    r = (u + add) & np.uint32(0xFFFFF000)
    return r.view(np.float32)


def _build_nc() -> bass.Bass:
    nc = bass.Bass()
    pt_d = nc.declare_dram_parameter("pt", [NB, 3, 128, W], F32R, isOutput=False)
    pcol_d = nc.declare_dram_parameter("pcol", [1, NB * N], F32R, isOutput=False)
    h0f_d = nc.declare_dram_parameter("h0f", [128, 3 * HID], F32R, isOutput=False)
    w1r_d = nc.declare_dram_parameter("w1r", [1, HID], F32R, isOutput=False)
    w2f_d = nc.declare_dram_parameter("w2f", [128, 2 * HID2], BF16, isOutput=False)
    out_ds = [
        nc.declare_dram_parameter(f"o{b:02d}", [1, N], F32, isOutput=True)
        for b in range(NB)
    ]

    ctx = ExitStack()
    with ctx:
        # ---- persistent SBUF ----
        h0f = ctx.enter_context(nc.sbuf_tensor("h0f_s", [128, 3 * HID], F32R))
        w1r = ctx.enter_context(nc.sbuf_tensor("w1r_s", [1, HID], F32R))
        w2f = ctx.enter_context(nc.sbuf_tensor("w2f_s", [128, 2 * HID2], BF16))
        pcall = ctx.enter_context(nc.sbuf_tensor("pcall_s", [1, NB * N], F32R))
        pt = [
            ctx.enter_context(nc.sbuf_tensor(f"ptb{s}", [128, 3 * W], F32R))
            for s in range(PTBUF)
        ]
        at = [
            ctx.enter_context(nc.sbuf_tensor(f"atb{s}", [128, 2 * N], BF16))
            for s in range(3)
        ]
        h2sb = [
            ctx.enter_context(nc.sbuf_tensor(f"h2b{s}", [128, N], F32R))
            for s in range(3)
        ]
        h3sb = [
            ctx.enter_context(nc.sbuf_tensor(f"h3b{s}", [128, W], F32R))
            for s in range(3)
        ]
        srow = [
            ctx.enter_context(nc.sbuf_tensor(f"srowb{s}", [1, N], F32))
            for s in range(SRBUF)
        ]
        # ---- PSUM: 8 banks exactly ----
        aps = [
            [
                ctx.enter_context(
                    nc.psum_tensor(f"apsb{p}{h}", [128, N], F32)
                )
                for h in range(2)
            ]
            for p in range(2)
        ]  # aps[pair][hc]
        h2ps = [
            ctx.enter_context(nc.psum_tensor(f"h2psb{s}", [128, N], F32))
            for s in range(2)
        ]
        h3ps = [
            ctx.enter_context(nc.psum_tensor(f"h3psb{s}", [128, W], F32))
            for s in range(2)
        ]

        # ---- semaphores ----
        sem_const = ctx.enter_context(nc.semaphore("sem_const"))
        sem_pt = [
            ctx.enter_context(nc.semaphore(f"sem_pt{s}")) for s in range(PTBUF)
        ]
        sem_out = [
            ctx.enter_context(nc.semaphore(f"sem_out{s}")) for s in range(SRBUF)
        ]
        sem_mm1 = ctx.enter_context(nc.semaphore("sem_mm1"))
        sem_relu = ctx.enter_context(nc.semaphore("sem_relu"))
        sem_mm2 = ctx.enter_context(nc.semaphore("sem_mm2"))
        sem_h2c = ctx.enter_context(nc.semaphore("sem_h2c"))
        sem_mm3 = ctx.enter_context(nc.semaphore("sem_mm3"))
        sem_h3c = ctx.enter_context(nc.semaphore("sem_h3c"))
        sem_mm4 = ctx.enter_context(nc.semaphore("sem_mm4"))
        sem_sc = ctx.enter_context(nc.semaphore("sem_sc"))

        block = ctx.enter_context(nc.Block())

        NI = NB + 3  # pipeline iterations (skew 3)

        @block.sync
        def _(sync):
            sync.dma_start(h0f[:, :], h0f_d[:, :]).then_inc(sem_const, 16)
            sync.dma_start(w1r[:, :], w1r_d[:, :]).then_inc(sem_const, 16)
            sync.dma_start(w2f[:, :], w2f_d[:, :]).then_inc(sem_const, 16)
            sync.dma_start(pcall[:, :], pcol_d[:, :]).then_inc(sem_const, 16)

        @block.gpsimd
        def _(g):
            for p in range(min(PTBUF, NB)):
                g.dma_start(
                    pt[p][:, :].rearrange("p (t w) -> p t w", t=3),
                    pt_d[p].rearrange("t p w -> p t w"),
                ).then_inc(sem_pt[p], 16)
            for i in range(NI):
                p = i + PTBUF
                if p < NB:
                    g.wait_ge(sem_mm3, i + 1)
                    g.dma_start(
                        pt[p % PTBUF][:, :].rearrange("p (t w) -> p t w", t=3),
                        pt_d[p].rearrange("t p w -> p t w"),
                    ).then_inc(sem_pt[p % PTBUF], 16)
                k = i - 3
                if k >= 0:
                    g.wait_ge(sem_sc, k + 1)
                    g.dma_start(out_ds[k][:, :], srow[k % SRBUF][:, :]).then_inc(
                        sem_out[k % SRBUF], 16
                    )

        @block.tensor
        def _(te):
            te.wait_ge(sem_const, 64)
            for i in range(NI):
                # ---- mm1(i): A_T chunks + rank-1, fp32r N=384 ----
                if i < NB:
                    if i >= 2:
                        te.wait_ge(sem_relu, 2 * (i - 2) + 2)  # aps pair reuse
                    te.wait_ge(sem_pt[i % PTBUF], 16 * (i // PTBUF + 1))
                    ptt = pt[i % PTBUF]
                    for hc in range(2):
                        dst = aps[i % 2][hc]
                        for t in range(3):
                            nc.tensor.matmul(
                                dst[:, :],
                                h0f[:, t * HID + hc * 128 : t * HID + hc * 128 + 128],
                                ptt[:, t * W : t * W + N],
                                start=(t == 0),
                                stop=False,
                            )
                        nc.tensor.matmul(
                            dst[:, :],
                            w1r[:, hc * 128 : (hc + 1) * 128],
                            pcall[:, i * N : (i + 1) * N],
                            start=False,
                            stop=True,
                        ).then_inc(sem_mm1, 1)
                # ---- mm2(i-1): h2 = A@W2, bf16 N=128 ----
                k = i - 1
                if 0 <= k < NB:
                    te.wait_ge(sem_relu, 2 * k + 2)
                    if k >= 2:
                        te.wait_ge(sem_h2c, k - 1)  # h2ps[k%2] reuse
                    dst = h2ps[k % 2]
                    for jc in range(3):
                        for ht in range(2):
                            mm = nc.tensor.matmul(
                                dst[:, jc * 128 : (jc + 1) * 128],
                                at[k % 3][
                                    :, ht * N + jc * 128 : ht * N + jc * 128 + 128
                                ],
                                w2f[:, ht * HID2 : (ht + 1) * HID2],
                                start=(ht == 0),
                                stop=(ht == 1),
                            )
                    mm.then_inc(sem_mm2, 1)
                # ---- mm3(i-2): h3T (+d col), fp32r N=385 ----
                k = i - 2
                if 0 <= k < NB:
                    te.wait_ge(sem_h2c, k + 1)
                    if k >= 2:
                        te.wait_ge(sem_h3c, k - 1)  # h3ps[k%2] reuse
                        te.wait_ge(sem_sc, k - 1)  # aliased S row was drained
                    dst = h3ps[k % 2]
                    ptt = pt[k % PTBUF]
                    for t in range(3):
                        mm = nc.tensor.matmul(
                            dst[:, :],
                            h2sb[k % 3][:, t * 128 : (t + 1) * 128],
                            ptt[:, t * W : (t + 1) * W],
                            start=(t == 0),
                            stop=(t == 2),
                        )
                    mm.then_inc(sem_mm3, 1)
                # ---- mm4(i-3): S row into h3ps[k%2] partition 0 ----
                k = i - 3
                if 0 <= k < NB:
                    te.wait_ge(sem_h3c, k + 1)
                    nc.tensor.matmul(
                        h3ps[k % 2][0:1, 0:N],
                        h3sb[k % 3][:, N : N + 1],
                        h3sb[k % 3][:, 0:N],
                        start=True,
                        stop=True,
                    ).then_inc(sem_mm4, 1)

        @block.scalar
        def _(sc):
            for i in range(NI):
                k = i
                if k < NB:
                    if k >= 3:
                        sc.wait_ge(sem_mm2, k - 2)  # at[k%3] reuse
                    for hc in range(2):
                        sc.wait_ge(sem_mm1, 2 * k + hc + 1)
                        nc.scalar.activation(
                            at[k % 3][:, hc * N : (hc + 1) * N],
                            aps[k % 2][hc][:, :],
                            AFT.Relu,
                        ).then_inc(sem_relu, 1)
                k = i - 3
                if 0 <= k < NB:
                    sc.wait_ge(sem_mm4, k + 1)
                    if k >= SRBUF:
                        sc.wait_ge(sem_out[k % SRBUF], 16 * (k // SRBUF))
                    nc.scalar.copy(
                        srow[k % SRBUF][:, :], h3ps[k % 2][0:1, 0:N]
                    ).then_inc(sem_sc, 1)

        @block.vector
        def _(ve):
            for i in range(NI):
                k = i - 1
                if 0 <= k < NB:
                    if k >= 3:
                        ve.wait_ge(sem_mm3, k - 2)  # h2sb[k%3] reuse
                    ve.wait_ge(sem_mm2, k + 1)
                    nc.vector.tensor_copy(
                        h2sb[k % 3][:, :], h2ps[k % 2][:, :]
                    ).then_inc(sem_h2c, 1)
                k = i - 2
                if 0 <= k < NB:
                    if k >= 3:
                        ve.wait_ge(sem_mm4, k - 2)  # h3sb[k%3] reuse
                    ve.wait_ge(sem_mm3, k + 1)
                    nc.vector.tensor_copy(
                        h3sb[k % 3][:, :], h3ps[k % 2][:, :]
                    ).then_inc(sem_h3c, 1)

    return nc


def _get_nc() -> bass.Bass:
    if "nc" not in _NC_CACHE:
        _NC_CACHE["nc"] = _build_nc()
    return _NC_CACHE["nc"]


def kernel(z, x, partials, W1, W2):
    global LAST_RESULT
    z = np.asarray(z, dtype=np.float32)
    x = np.asarray(x, dtype=np.float32)
    partials = np.asarray(partials, dtype=np.float32)
    W1 = np.asarray(W1, dtype=np.float32)
    W2 = np.asarray(W2, dtype=np.float32)

    H0 = z[0] @ W1[:D]  # [384, 256]
    h0f = _round_f32r(
        np.ascontiguousarray(H0.reshape(3, 128, HID).transpose(1, 0, 2)).reshape(
            128, 3 * HID
        )
    )
    w1r = _round_f32r(np.ascontiguousarray(W1[D : D + 1]))  # [1, 256]
    w2f = (
        np.ascontiguousarray(W2.reshape(2, 128, HID2).transpose(1, 0, 2))
        .reshape(128, 2 * HID2)
        .astype(ml_dtypes.bfloat16)
    )

    ptT = np.ascontiguousarray(partials.transpose(0, 2, 1))  # ptT[g,j,i]=P_g[i,j]
    ar = np.arange(N)
    prow = partials[ar, ar, :]  # [384, 384]  P_g[g, :]
    pcol = ptT[ar, ar, :]  # [384, 384]  P_g[:, g]

    in_maps = []
    for c in range(NCORES):
        gs = slice(c * NB, (c + 1) * NB)
        aug = np.zeros((NB, 3, 128, W), dtype=np.float32)
        aug[..., :N] = ptT[gs].reshape(NB, 3, 128, N)
        aug[..., N] = prow[gs].reshape(NB, 3, 128)
        in_maps.append(
            {
                "pt": _round_f32r(aug),
                "pcol": _round_f32r(np.ascontiguousarray(pcol[gs])).reshape(1, NB * N),
                "h0f": h0f,
                "w1r": w1r,
                "w2f": w2f,
            }
        )

    nc = _get_nc()
    res = run_bass_kernel_spmd(
        nc,
        in_maps,
        core_ids=list(range(NCORES)),
        trace=bool(os.environ.get("KERNEL_TRACE")),
    )
    LAST_RESULT = res
    S = np.concatenate(
        [
            np.concatenate(
                [
                    np.asarray(res.results[c][f"o{b:02d}"], np.float32)
                    for b in range(NB)
                ],
                axis=0,
            )
            for c in range(NCORES)
        ],
        axis=0,
    )  # [384, 384] raw supplement rows
    sup = np.tril(S)
    sup = (sup + sup.T) * np.float32(0.5)
    return (x + sup).astype(np.float32)
